# revision 15
# baseline (speedup 1.0000x reference)
"""Trainium2 Bass kernel for nn_Net_12816182411419 (gnn_message_passing).

Model (reference):
    3x GraphConv(4096->4096) with norm='both' + ReLU  (54-node graph, 288 edges)
    global MLP 64->16->16->64 (tiny)
    out = sigmoid(relu(concat(embeds, g) @ Wo1 + bo1) @ Wo2 + bo2)

Strategy (8 NeuronCores, memory-bound: ~276MB fp32 of weights, streamed as
fp8e4m3 = ~8.8MB per core):
  - The graph scatter/gather is folded on the host into a dense 54x54 matrix
    M = diag(norm_in) @ A @ diag(norm_out), so each layer is
    X_next = relu((M @ X) @ W + b).
  - ALL THREE layers are column-sharded (core c owns output features
    [c*512,(c+1)*512)).  The inter-layer handoff is a small AllGather:
    after each layer a core mixes its own shard (M @ X_shard) via a
    matmul against M^T -- which directly produces the TRANSPOSED lhsT
    chunks the next layer needs -- and AllGathers the [512, 54] bf16
    chunk block (55KB/rank) instead of AllReducing a [54,4096] fp32
    partial (884KB, ~55us observed).
  - Weights are stored fp8e4m3 scaled by 256 (Wo1 by 8192; biases by 256
    and folded into the matmul as a K=1 contraction row of a bf16 ones
    vector); activations stay bf16 (mixed-dtype matmul).  The unscale is
    fused into the relu as one DVE tensor_scalar (mult 1/256, max 0).
  - All weights live resident in SBUF and are streamed up-front on the
    sync HWDGE ring in consumption order; activations, consts and
    AllGather staging ride the scalar HWDGE ring.
  - A few dummy matmuls on zeroed SBUF warm the PE HAM clock gate during
    the initial weight-stream window so layer 1 runs at 2.4GHz.
  - Row-shard Wo1 (the [54*4096, 85] output layer) to match the layer-3
    shard; each core computes partial [85]s (4-way PE column-tiled matvec)
    which the host reduces.
  - Tiny global-MLP / Wo2 / final sigmoid run on the host.

kernel(**inputs) takes the FULL reference inputs and returns the FULL output.
"""

import os
import sys

# The device path needs the axon jax platform; undo a cpu pin if a caller set
# one before we got here (reference code wants cpu jax, but we never use jax).
if os.environ.get("JAX_PLATFORMS") == "cpu":
    os.environ.pop("JAX_PLATFORMS")

import ml_dtypes
import numpy as np

import concourse.bass as bass
import concourse.mybir as mybir
import concourse.tile as tile
from concourse import bacc
from concourse.bass import ds
from concourse.bass_utils import run_bass_kernel_spmd

# ---------------------------------------------------------------------------
# NTFF profile hook injection (axon container ships an antenv stub without
# axon_hooks; provide it so run_bass_kernel_spmd(trace=True) can profile).
# Best-effort: kernel correctness never depends on this.
try:
    import types

    import antenv

    if not hasattr(antenv, "axon_hooks"):
        _hooks_mod = types.ModuleType("antenv.axon_hooks")
        _hooks_mod._hook = None

        def _set_hook(h):
            _hooks_mod._hook = h

        def _get_hook():
            return _hooks_mod._hook

        _hooks_mod.set_axon_ntff_profile_hook = _set_hook
        _hooks_mod.get_axon_ntff_profile_hook = _get_hook
        sys.modules["antenv.axon_hooks"] = _hooks_mod
        antenv.axon_hooks = _hooks_mod
        try:
            from trn_agent_boot.trn_boot import _ntff_profile_via_ctypes

            _hook = _ntff_profile_via_ctypes("/opt/axon/libaxon_pjrt.so")
            if _hook is not None:
                _set_hook(_hook)
        except Exception:
            pass
except Exception:
    pass
# ---------------------------------------------------------------------------

N = 54          # nodes
D = 4096        # feature dim
NCORES = 8
S = D // NCORES  # 512 per-core feature shard
P = 128
KC = D // P      # 32 contraction chunks
SC = S // P      # 4 chunks within a shard
E = 85           # output-layer width
TPB = N * S // P  # 216 wo1 contraction chunks per core
WOB = 36         # wo chunks per DMA slice (6 slices)
W1SYNC = [4, 4, 8]          # kc per w1 DMA slice on the sync ring
W1SCAL = [8, 8]             # kc per w1 DMA slice on the scalar ring
NWARM = 5        # PE HAM warm-up dummy matmuls

A_NP = ml_dtypes.bfloat16       # activation dtype (host side)
W_NP = ml_dtypes.float8_e4m3fn  # weight dtype (host side)
G_NP = ml_dtypes.float8_e4m3fn  # AllGather payload dtype (host side)
A_DT = mybir.dt.bfloat16
W_DT = mybir.dt.float8e4
G_DT = mybir.dt.float8e4
WSCALE = 256.0              # host-side weight scale (relu unscales)
WOSCALE = 8192.0            # host-side Wo1 scale (host divides partials)


def _emit_kernel(tc, x0t, mt, i54, w1, w2, w3, b1, b2, b3, wo, out):
    nc = tc.nc
    f32 = mybir.dt.float32
    mult = mybir.AluOpType.mult
    maxop = mybir.AluOpType.max

    with (
        tc.tile_pool(name="consts", bufs=1) as consts,
        tc.tile_pool(name="w1p", bufs=len(W1SYNC) + len(W1SCAL)) as w1p,
        tc.tile_pool(name="w2p", bufs=4) as w2p,
        tc.tile_pool(name="w3p", bufs=4) as w3p,
        tc.tile_pool(name="wop", bufs=6) as wop,
        tc.tile_pool(name="xpt", bufs=2) as xptp,
        tc.tile_pool(name="stage", bufs=3) as stp,
        tc.tile_pool(name="pwarm", bufs=1, space="PSUM") as pwp,
        tc.tile_pool(name="py", bufs=2, space="PSUM") as pyp,
        tc.tile_pool(name="pmix", bufs=2, space="PSUM") as pmixp,
        tc.tile_pool(name="po", bufs=1, space="PSUM") as pop,
        tc.tile_pool(name="dram", bufs=1, space="DRAM") as dramp,
    ):
        # -------- all weight streams issued up-front on the sync HWDGE ring
        # in consumption order; everything is SBUF-resident (no reuse), so
        # the DMA engines never stall on compute.  w1's tail rides the
        # scalar ring in parallel so layer 1's weights land by ~10us.
        w1ts, w2ts, w3ts, wots = [], [], [], []
        w1map = []  # kc -> (slice_idx, local_idx)
        off = 0
        for si, nk in enumerate(W1SYNC + W1SCAL):
            t = w1p.tile([P, nk, S], W_DT, tag="w1t", name=f"w1t{si}")
            if si < len(W1SYNC):
                nc.sync.dma_start(t[:], w1[:, ds(off, nk), :])
            w1ts.append(t)
            for j in range(nk):
                w1map.append((si, j))
            off += nk
        for g in range(4):
            t = w2p.tile([P, 8, S], W_DT, tag="w2t", name=f"w2t{g}")
            nc.sync.dma_start(t[:], w2[:, ds(8 * g, 8), :])
            w2ts.append(t)
        for g in range(4):
            t = w3p.tile([P, 8, S], W_DT, tag="w3t", name=f"w3t{g}")
            nc.sync.dma_start(t[:], w3[:, ds(8 * g, 8), :])
            w3ts.append(t)
        for sblk in range(TPB // WOB):
            t = wop.tile([P, WOB, E], W_DT, tag="wot", name=f"wot{sblk}")
            nc.sync.dma_start(t[:], wo[:, ds(WOB * sblk, WOB), :])
            wots.append(t)

        # -------- consts + layer-1 lhsT + w1 tail on the scalar HWDGE ring
        # (x0t split so layer 1's first matmuls are not gated on the full
        # 442KB transfer)
        xpt1 = xptp.tile([P, KC, N], A_DT, tag="xpt")
        for g in range(4):
            nc.scalar.dma_start(xpt1[:, ds(8 * g, 8), :], x0t[:, ds(8 * g, 8), :])
        mt_t = consts.tile([P, N], A_DT, tag="mt")
        nc.scalar.dma_start(mt_t[:], mt)
        i54_t = consts.tile([P, N], A_DT, tag="i54")
        nc.scalar.dma_start(i54_t[:], i54)
        b1_t = consts.tile([1, S], W_DT, tag="b1")
        nc.scalar.dma_start(b1_t[:], b1)
        off = sum(W1SYNC)
        for si, nk in enumerate(W1SCAL):
            nc.scalar.dma_start(
                w1ts[len(W1SYNC) + si][:], w1[:, ds(off, nk), :]
            )
            off += nk
        b2_t = consts.tile([1, S], W_DT, tag="b2")
        nc.scalar.dma_start(b2_t[:], b2)
        b3_t = consts.tile([1, S], W_DT, tag="b3")
        nc.scalar.dma_start(b3_t[:], b3)

        # persistent activation tile: nodes on partitions; pad rows 54..127
        # zeroed once (mix/transpose matmuls read all 128 partitions; the
        # zero-padded mt/i54 rhs kills them, but garbage could be Inf/NaN).
        xsr = consts.tile([P, S], A_DT, tag="xsr")
        nc.any.memzero(xsr[:])
        ones = consts.tile([1, N], A_DT, tag="ones")
        nc.vector.memset(ones[:], 1.0)

        # PE HAM warm-up: dummy matmuls on the zeroed tiles while the first
        # w1 slices stream in, so layer 1 runs un-throttled at 2.4GHz.
        pw = pwp.tile([P, S], f32, tag="pw")
        for _ in range(NWARM):
            nc.tensor.matmul(
                pw[:], xsr[:, ds(0, P)], xsr[:], start=True, stop=True,
                skip_group_check=True,
            )

        def layer(chunk, wts, wmap, b_t):
            py = pyp.tile([N, S], f32, tag="py")
            for kc in range(KC):
                si, j = wmap[kc]
                nc.tensor.matmul(
                    py[:], chunk(kc), wts[si][:, j, :],
                    start=(kc == 0), stop=False,
                )
            # bias folded in as a K=1 contraction row (ones^T @ b_row),
            # last so the bias DMA never gates the layer start
            nc.tensor.matmul(py[:], ones[:], b_t[:], start=False, stop=True)
            # xsr[:N] = relu(py / WSCALE), cast to A_DT, fused on DVE
            nc.vector.tensor_scalar(
                out=xsr[:N, :], in0=py[:],
                scalar1=1.0 / WSCALE, scalar2=0.0, op0=mult, op1=maxop,
            )

        w8map = [(kc // 8, kc % 8) for kc in range(KC)]

        def mixT(rhs_t, tag):
            # chunk fb of (M @ X_shard)^T: [128(f), 54(n)] lhsT blocks;
            # all 4 chunks land in one PSUM bank -> one copy -> one stage.
            pm = pmixp.tile([P, SC, N], f32, tag="pmix")
            for fb in range(SC):
                nc.tensor.matmul(
                    pm[:, fb, :], xsr[:, ds(fb * P, P)], rhs_t[:],
                    start=True, stop=True, skip_group_check=True,
                )
            mst = stp.tile([P, SC, N], G_DT, tag="mst", name=f"mst{tag}")
            nc.any.tensor_copy(out=mst[:], in_=pm[:])
            return mst

        def gather(mst, tag):
            agi = dramp.tile([P, SC * N], G_DT, tag=f"agi{tag}", name=f"agi{tag}")
            nc.scalar.dma_start(agi[:], mst.rearrange("p f n -> p (f n)"))
            ago = dramp.tile(
                [NCORES * P, SC * N], G_DT, tag=f"ago{tag}", name=f"ago{tag}"
            )
            nc.gpsimd.collective_compute(
                "AllGather",
                mybir.AluOpType.bypass,
                replica_groups=[list(range(NCORES))],
                ins=[agi.opt()],
                outs=[ago.opt()],
            )
            xpt = xptp.tile([P, KC, N], G_DT, tag="xptg", name=f"xpt{tag}")
            xv = xpt.rearrange("p (c f) n -> p c f n", c=NCORES)
            av = ago.rearrange("(c p) (f n) -> p c f n", p=P, n=N)
            for h in range(4):
                nc.scalar.dma_start(xv[:, ds(2 * h, 2)], av[:, ds(2 * h, 2)])
            return xpt

        # ---------------- layer 1 (lhsT direct from host) -> AG -> layer 2
        layer(lambda kc: xpt1[:, kc, :], w1ts, w1map, b1_t)
        xpt2 = gather(mixT(mt_t, "1"), "1")
        layer(lambda kc: xpt2[:, kc, :], w2ts, w8map, b2_t)
        xpt3 = gather(mixT(mt_t, "2"), "2")
        layer(lambda kc: xpt3[:, kc, :], w3ts, w8map, b3_t)
        # transpose X3 shard to feature-major (no mix after layer 3)
        xt3 = mixT(i54_t, "3")

        # ---------------- output layer partial: 216 chunks of
        # (lhsT [128,1], rhs [128,85]), 4-way column-tiled across PE groups.
        po = pop.tile([P, E], f32, tag="po")
        for t in range(TPB):
            n, fb = t // SC, t % SC
            nc.tensor.matmul(
                po[ds(32 * fb, 1), :],
                xt3[:, fb, ds(n, 1)],
                wots[t // WOB][:, t % WOB, :],
                start=(n == 0),
                stop=(n == N - 1),
                tile_position=(0, 32 * fb),
                skip_group_check=True,
            )
        osb = consts.tile([P, E], f32, tag="osb")
        for fb in range(SC):
            nc.any.tensor_copy(
                out=osb[ds(32 * fb, 1), :], in_=po[ds(32 * fb, 1), :]
            )
        nc.scalar.dma_start(out, osb.rearrange("(j r) e -> j r e", j=4)[:, 0, :])


_NC_CACHE = {}


def _build_nc():
    if "nc" in _NC_CACHE:
        return _NC_CACHE["nc"]
    nc = bacc.Bacc(
        "TRN2", target_bir_lowering=False, debug=False, num_devices=NCORES
    )
    f32 = mybir.dt.float32
    x0t = nc.dram_tensor("x0t", [P, KC, N], A_DT, kind="ExternalInput").ap()
    mt = nc.dram_tensor("mt", [P, N], A_DT, kind="ExternalInput").ap()
    i54 = nc.dram_tensor("i54", [P, N], A_DT, kind="ExternalInput").ap()
    w1 = nc.dram_tensor("w1", [P, KC, S], W_DT, kind="ExternalInput").ap()
    w2 = nc.dram_tensor("w2", [P, KC, S], W_DT, kind="ExternalInput").ap()
    w3 = nc.dram_tensor("w3", [P, KC, S], W_DT, kind="ExternalInput").ap()
    b1 = nc.dram_tensor("b1", [1, S], W_DT, kind="ExternalInput").ap()
    b2 = nc.dram_tensor("b2", [1, S], W_DT, kind="ExternalInput").ap()
    b3 = nc.dram_tensor("b3", [1, S], W_DT, kind="ExternalInput").ap()
    wo = nc.dram_tensor("wo", [P, TPB, E], W_DT, kind="ExternalInput").ap()
    out = nc.dram_tensor("out", [4, E], f32, kind="ExternalOutput").ap()

    with tile.TileContext(nc) as tc:
        _emit_kernel(tc, x0t, mt, i54, w1, w2, w3, b1, b2, b3, wo, out)
    nc.compile()
    _NC_CACHE["nc"] = nc
    return nc


def _pack_w(Wfull, c):
    """[4096, 512] column shard -> [128, 32, 512] rhs tiles, scaled+cast."""
    shard = np.asarray(Wfull, np.float32)[:, c * S : (c + 1) * S] * WSCALE
    return np.ascontiguousarray(
        shard.reshape(KC, P, S).transpose(1, 0, 2).astype(W_NP)
    )


def _host_prep(inputs):
    """Build per-core device input maps + host-side tail closure."""
    feat = np.asarray(inputs["feat"], np.float32)
    globalFeats = np.asarray(inputs["globalFeats"], np.float32)
    src = np.asarray(inputs["src"], np.int64)
    dst = np.asarray(inputs["dst"], np.int64)

    # Dense folded graph operator M = diag(norm_in) @ A @ diag(norm_out)
    A = np.zeros((N, N), np.float64)
    np.add.at(A, (dst, src), 1.0)
    deg_out = np.bincount(src, minlength=N).astype(np.float64)
    deg_in = np.bincount(dst, minlength=N).astype(np.float64)
    norm_out = 1.0 / np.sqrt(np.maximum(deg_out, 1.0))
    norm_in = 1.0 / np.sqrt(np.maximum(deg_in, 1.0))
    M = (norm_in[:, None] * A * norm_out[None, :]).astype(np.float32)

    # Layer-1 lhsT: (M @ feat)^T packed as [128, 32, 54]
    x0p = (M.astype(np.float64) @ feat.astype(np.float64)).astype(np.float32)
    x0t = np.ascontiguousarray(
        x0p.T.reshape(KC, P, N).transpose(1, 0, 2).astype(A_NP)
    )

    mt_pad = np.zeros((P, N), np.float32)
    mt_pad[:N, :] = M.T
    i54_pad = np.zeros((P, N), np.float32)
    i54_pad[:N, :] = np.eye(N, dtype=np.float32)
    mt_pad = mt_pad.astype(A_NP)
    i54_pad = i54_pad.astype(A_NP)

    Wo1 = np.asarray(inputs["Wo1"], np.float32)
    Wo1_emb = Wo1[: N * D].reshape(N, D, E)
    b1 = np.asarray(inputs["b1"], np.float32) * WSCALE
    b2 = np.asarray(inputs["b2"], np.float32) * WSCALE
    b3 = np.asarray(inputs["b3"], np.float32) * WSCALE

    in_maps = []
    for c in range(NCORES):
        sl = slice(c * S, (c + 1) * S)
        m = {
            "x0t": x0t,
            "mt": mt_pad,
            "i54": i54_pad,
            "w1": _pack_w(inputs["W1"], c),
            "w2": _pack_w(inputs["W2"], c),
            "w3": _pack_w(inputs["W3"], c),
            "b1": np.ascontiguousarray(b1[sl].reshape(1, S).astype(W_NP)),
            "b2": np.ascontiguousarray(b2[sl].reshape(1, S).astype(W_NP)),
            "b3": np.ascontiguousarray(b3[sl].reshape(1, S).astype(W_NP)),
        }
        shard = Wo1_emb[:, sl, :].reshape(N * S, E) * WOSCALE
        m["wo"] = np.ascontiguousarray(
            shard.reshape(TPB, P, E).transpose(1, 0, 2).astype(W_NP)
        )
        in_maps.append(m)

    # Host tail: global MLP + bias + relu + Wo2 + sigmoid
    def finish(partials):
        total = np.zeros(E, np.float64)
        for p in partials:
            total += p.astype(np.float64).sum(axis=0) / WOSCALE
        g = np.maximum(
            globalFeats @ np.asarray(inputs["Wg1"], np.float32)
            + np.asarray(inputs["bg1"], np.float32),
            0.0,
        )
        g = np.maximum(
            g @ np.asarray(inputs["Wg2"], np.float32)
            + np.asarray(inputs["bg2"], np.float32),
            0.0,
        )
        g = np.maximum(
            g @ np.asarray(inputs["Wg3"], np.float32)
            + np.asarray(inputs["bg3"], np.float32),
            0.0,
        )
        total += g.astype(np.float64) @ Wo1[N * D :].astype(np.float64)
        total += np.asarray(inputs["bo1"], np.float32).astype(np.float64)
        out_vec = np.maximum(total, 0.0).astype(np.float32)
        y = out_vec @ np.asarray(inputs["Wo2"], np.float32) + np.asarray(
            inputs["bo2"], np.float32
        )
        return (1.0 / (1.0 + np.exp(-y))).astype(np.float32)

    return in_maps, finish


def kernel_with_results(inputs, trace=False, trace_cores=None):
    nc = _build_nc()
    in_maps, finish = _host_prep(inputs)
    results = run_bass_kernel_spmd(
        nc,
        in_maps,
        core_ids=list(range(NCORES)),
        trace=trace,
        trace_cores=trace_cores,
    )
    partials = [r["out"] for r in results.results]
    return finish(partials), results


def kernel(**inputs):
    out, _ = kernel_with_results(inputs, trace=False)
    return out
